# revision 1
# baseline (speedup 1.0000x reference)
"""Trainium2 Bass kernel for prefix-LM CausalSelfAttention.

Problem: B=2, T=2048, C=2048, H=16 heads (hd=128), prefix-LM mask
(bidirectional over first half, causal after), RoPE on q/k.

Sharding over 8 cores: data-parallel on batch (2) x tensor-parallel on
heads (4 heads per core). Each core computes a partial output projection
(its heads' contribution); partials are summed on host.

Per-core dataflow (all matmuls in float32r: full PE rate, ~1e-4 rel err):
  1. qT/kT = W^T @ x^T    [hd*4, T] "transposed" layout (head-major tiles)
  2. RoPE via pair-swap permutation matmul + DVE combine with cos/sin planes
  3. v = x @ Wv           [T, hd*4] natural layout
  4. Per head, per 512-wide query chunk I, over unmasked 128-key tiles J:
       S'[J] = k_rope[:,J]^T-tile x q_rope[:,I]      (scores transposed, [j,i])
       P'[J] = exp(S' * 1/sqrt(hd))                  (ACT, PSUM->SBUF, f32r)
       mask-multiply for diagonal-crossing tiles only (4 static patterns)
       y_psum  += v[J,h]^T-as-lhsT x P'[J]           (PV, out y^T [hd, i])
       d_psum  += ones^T x P'[J]                     (softmax denominator)
     y^T[:, I] = y_psum * broadcast(1/d)             (normalize, f32r)
  5. partial_out = y^T-as-lhsT x Wp  accumulated over the 4 heads.

Fully-masked key tiles are skipped (structural sparsity: 44/64 tiles/head).
"""
import math

import numpy as np

N_HEAD = 16
B = 2
T = 2048
C = 2048
HD = 128
HPC = 4          # heads per core
CL = HPC * HD    # local C = 512
TC = 512         # chunk width (matmul moving free dim / psum bank)
NT = T // TC     # 4 chunks
KT = C // 128    # 16 contraction tiles over C
TT = T // 128    # 16 T tiles
SCALE = 1.0 / math.sqrt(HD)

# Per query-chunk I: list of (J, mask_idx) key tiles to compute.
# mask_idx is None for fully-allowed tiles, else 0..3 selecting the
# static diagonal pattern mask[d][jj, ii] = (ii >= jj + 128*d).
_JLISTS = {
    0: [(j, None) for j in range(8)],
    1: [(j, None) for j in range(8)],
    2: [(j, None) for j in range(8)] + [(8 + d, d) for d in range(4)],
    3: [(j, None) for j in range(12)] + [(12 + d, d) for d in range(4)],
}

_CACHE = {}


def _build_nc():
    import concourse.tile as tile
    import concourse.mybir as mybir
    from concourse import bacc

    f32 = mybir.dt.float32
    f32r = mybir.dt.float32r

    nc = bacc.Bacc(None, target_bir_lowering=False)

    xT = nc.dram_tensor("xT", [C, T], f32r, kind="ExternalInput")
    wqk = nc.dram_tensor("wqk", [C, 2 * CL], f32r, kind="ExternalInput")
    wv = nc.dram_tensor("wv", [C, CL], f32r, kind="ExternalInput")
    wp = nc.dram_tensor("wp", [CL, C], f32r, kind="ExternalInput")
    cosP = nc.dram_tensor("cosP", [HD, T], f32, kind="ExternalInput")
    sinP = nc.dram_tensor("sinP", [HD, T], f32, kind="ExternalInput")
    rt = nc.dram_tensor("rt", [HD, HD], f32r, kind="ExternalInput")
    masks = nc.dram_tensor("masks", [4, 128, TC], f32r, kind="ExternalInput")
    ones = nc.dram_tensor("ones", [128, 1], f32r, kind="ExternalInput")
    out = nc.dram_tensor("out", [T, C], f32, kind="ExternalOutput")

    xT3 = xT.rearrange("(kt p) t -> p kt t", p=128)
    wqk3 = wqk.rearrange("(kt p) m -> p kt m", p=128)
    wv3 = wv.rearrange("(kt p) m -> p kt m", p=128)
    wp3 = wp.rearrange("(kt p) m -> p kt m", p=128)
    masks3 = masks.rearrange("d p n -> p d n")

    Exp = mybir.ActivationFunctionType.Exp

    with tile.TileContext(nc) as tc:
        # Pools are alloc'd/released manually (non-LIFO lifetimes) so DMA
        # prefetch for the next phase can be issued while the previous
        # phase's working set is still alive. Budget ~208 KB/partition.
        mpool = tc.alloc_tile_pool(name="misc", bufs=1)            # 9K whole
        qk_pool = tc.alloc_tile_pool(name="qkrope", bufs=1)        # 64K whole
        tpool = tc.alloc_tile_pool(name="trig", bufs=1, side="right")  # 16K A..rope

        rt_sb = mpool.tile([HD, HD], f32r)
        ones_sb = mpool.tile([128, 1], f32r)
        mask_sb = mpool.tile([128, 4, TC], f32r)
        cos_sb = tpool.tile([HD, T], f32)
        sin_sb = tpool.tile([HD, T], f32)

        # qkT[m] for m in 0..7: m<4 -> q head m, else k head m-4; [hd, T]
        # (rope outputs later reuse the same slots via identical tags)
        qkT = [qk_pool.tile([128, T], f32r, tag=f"qk{m}", name=f"qk{m}") for m in range(8)]

        # ---- stage A: qT/kT = W_{q,k}^T @ x^T, head-major tiles ----
        wpool = tc.alloc_tile_pool(name="wqk_sb", bufs=1)          # 64K A
        xpool = tc.alloc_tile_pool(name="xt_qk", bufs=1)           # 44K A
        ps1 = tc.alloc_tile_pool(name="ps_qk", bufs=4, space="PSUM")
        # interleave W / first-chunk x DMAs so PE can start accumulating
        # group (m=0, n=0) as soon as w[0]/x[0] land
        w_t = []
        x_first = []
        for k in range(KT):
            wt = wpool.tile([128, 2 * CL], f32r, tag=f"w{k}", name=f"w{k}")
            nc.sync.dma_start(out=wt, in_=wqk3[:, k])
            w_t.append(wt)
            xt = xpool.tile([128, TC], f32r, tag=f"x{k}", name=f"x{k}",
                            bufs=2 if k < 6 else 1)
            nc.sync.dma_start(out=xt, in_=xT3[:, k, 0:TC])
            x_first.append(xt)
        nc.sync.dma_start(out=rt_sb, in_=rt[:, :])
        nc.sync.dma_start(out=cos_sb, in_=cosP[:, :])
        nc.sync.dma_start(out=sin_sb, in_=sinP[:, :])
        for n in range(NT):
            if n == 0:
                x_t = x_first
            else:
                x_t = []
                for k in range(KT):
                    xt = xpool.tile([128, TC], f32r, tag=f"x{k}", name=f"x{k}",
                                    bufs=2 if k < 6 else 1)
                    nc.sync.dma_start(out=xt, in_=xT3[:, k, n * TC:(n + 1) * TC])
                    x_t.append(xt)
            for m in range(8):
                ps = ps1.tile([128, TC], f32, tag="ps_qk", name="ps_qk")
                for k in range(KT):
                    nc.tensor.matmul(
                        ps, w_t[k][:, m * 128:(m + 1) * 128], x_t[k],
                        start=(k == 0), stop=(k == KT - 1),
                    )
                nc.vector.tensor_copy(out=qkT[m][:, n * TC:(n + 1) * TC], in_=ps)
        xpool.release()
        wpool.release()
        ps1.release()

        # ---- stage B: RoPE on q and k (outputs reuse qk slots) ----
        # rope = qkT*cos + (R @ qkT)*sin ; R = pair swap w/ sign.
        # m-order (0,4,1,5,..) so head 0's q/k finish first and attention
        # can start while later heads still rope. v-phase DMAs (wv, xv)
        # are issued up front so v matmuls overlap RoPE's DVE work.
        v_pool = tc.alloc_tile_pool(name="v_sb", bufs=1)           # 32K ..attn
        xvpool = tc.alloc_tile_pool(name="xt_v", bufs=2)           # 32K ..v
        wvpool = tc.alloc_tile_pool(name="wv_sb", bufs=1)          # 32K ..v
        v_t = [v_pool.tile([128, CL], f32r, tag=f"v{mt}", name=f"v{mt}")
               for mt in range(TT)]
        wv_t = []
        for k in range(KT):
            wt = wvpool.tile([128, CL], f32r, tag=f"wv{k}", name=f"wv{k}")
            nc.sync.dma_start(out=wt, in_=wv3[:, k])
            wv_t.append(wt)
        xv_t = {}
        for pair in range(TT // 2):
            for k in range(KT):
                xt = xvpool.tile([128, 256], f32r, tag=f"xv{k}", name=f"xv{k}")
                nc.sync.dma_start(
                    out=xt, in_=xT3[:, k, pair * 256:(pair + 1) * 256]
                )
                xv_t[(pair, k)] = xt

        rope = [None] * 8
        rtmp = tc.alloc_tile_pool(name="rope_tmp", bufs=4)
        psr = tc.alloc_tile_pool(name="ps_rot", bufs=4, space="PSUM")
        for m in (0, 4, 1, 5, 2, 6, 3, 7):
            tmp = []
            for n in range(NT):
                sl = slice(n * TC, (n + 1) * TC)
                ps = psr.tile([128, TC], f32, tag="ps_rot", name="ps_rot")
                nc.tensor.matmul(ps, rt_sb, qkT[m][:, sl], start=True, stop=True)
                t1 = rtmp.tile([128, TC], f32, tag="t1", name="t1")
                t2 = rtmp.tile([128, TC], f32, tag="t2", name="t2")
                nc.vector.tensor_mul(t1, ps, sin_sb[:, sl])
                nc.vector.tensor_mul(t2, qkT[m][:, sl], cos_sb[:, sl])
                tmp.append((t1, t2))
            # all reads of qkT[m] issued; now write into its slot
            ro = qk_pool.tile([128, T], f32r, tag=f"qk{m}", name=f"rope{m}")
            for n in range(NT):
                sl = slice(n * TC, (n + 1) * TC)
                nc.vector.tensor_add(ro[:, sl], tmp[n][0], tmp[n][1])
            rope[m] = ro
        rtmp.release()
        psr.release()
        tpool.release()

        # ---- stage C: v = x @ Wv (natural layout), all 4 heads ----
        ps2 = tc.alloc_tile_pool(name="ps_v", bufs=4, space="PSUM")
        nc.sync.dma_start(out=ones_sb, in_=ones[:, :])
        nc.sync.dma_start(out=mask_sb, in_=masks3)
        for pair in range(TT // 2):
            for half in range(2):
                mt = 2 * pair + half
                ps = ps2.tile([128, CL], f32, tag="ps_v", name="ps_v")
                for k in range(KT):
                    nc.tensor.matmul(
                        ps, xv_t[(pair, k)][:, half * 128:(half + 1) * 128],
                        wv_t[k], start=(k == 0), stop=(k == KT - 1),
                    )
                nc.vector.tensor_copy(out=v_t[mt], in_=ps)
        wvpool.release()
        xvpool.release()
        ps2.release()

        # ---- stage D: attention; stage E (proj) overlaps its tail ----
        y_pool = tc.alloc_tile_pool(name="yT_sb", bufs=1)          # 32K
        yT = [y_pool.tile([128, T], f32r, tag=f"yT{h}", name=f"yT{h}")
              for h in range(HPC)]
        wppool = tc.alloc_tile_pool(name="wp_sb", bufs=1)          # 32K
        wp_t = []
        for hk in range(HPC):
            wt = wppool.tile([128, C], f32r, tag=f"wp{hk}", name=f"wp{hk}")
            nc.sync.dma_start(out=wt, in_=wp3[:, hk])
            wp_t.append(wt)

        pp_pool = tc.alloc_tile_pool(name="pp", bufs=5)
        sm_pool = tc.alloc_tile_pool(name="small", bufs=2)
        ps_s = tc.alloc_tile_pool(name="ps_s", bufs=2, space="PSUM")
        ps_y = tc.alloc_tile_pool(name="ps_y", bufs=2, space="PSUM")
        ps_d = tc.alloc_tile_pool(name="ps_d", bufs=2, space="PSUM")
        ps_o = tc.alloc_tile_pool(name="ps_o", bufs=2, space="PSUM")
        opool = tc.alloc_tile_pool(name="ostage", bufs=3)

        for h in range(HPC):
            q_h = rope[h]
            k_h = rope[4 + h]
            for I in range(NT):
                isl = slice(I * TC, (I + 1) * TC)
                jl = _JLISTS[I]
                y_ps = ps_y.tile([128, TC], f32, tag="y", name="y_ps")
                d_ps = ps_d.tile([1, TC], f32, tag="d", name="d_ps")
                for jidx, (J, d) in enumerate(jl):
                    s_ps = ps_s.tile([128, TC], f32, tag="s", name="s_ps")
                    nc.tensor.matmul(
                        s_ps, k_h[:, J * 128:(J + 1) * 128],
                        q_h[:, isl], start=True, stop=True,
                    )
                    pp = pp_pool.tile([128, TC], f32r, tag="pp", name="pp")
                    nc.scalar.activation(out=pp, in_=s_ps, func=Exp, scale=SCALE)
                    if d is not None:
                        ppm = pp_pool.tile([128, TC], f32r, tag="ppm",
                                           name="ppm", bufs=2)
                        nc.vector.tensor_mul(ppm, pp, mask_sb[:, d])
                        pp = ppm
                    first = jidx == 0
                    last = jidx == len(jl) - 1
                    nc.tensor.matmul(
                        y_ps, v_t[J][:, h * 128:(h + 1) * 128], pp,
                        start=first, stop=last,
                    )
                    nc.tensor.matmul(d_ps, ones_sb, pp, start=first, stop=last)
                recip = sm_pool.tile([1, TC], f32, tag="recip", name="recip")
                nc.vector.reciprocal(out=recip, in_=d_ps)
                recipB = sm_pool.tile([128, TC], f32, tag="recipB", name="recipB")
                nc.gpsimd.partition_broadcast(recipB, recip)
                nc.vector.tensor_mul(yT[h][:, isl], y_ps, recipB)

        # ---- stage E: partial out = yT^T @ Wp, grouped by query chunk so
        # chunks whose yT rows are complete overlap the remaining attention
        for I in range(NT):
            for ml in range(4):
                mt = 4 * I + ml
                msl = slice(mt * 128, (mt + 1) * 128)
                for n in range(NT):
                    ps = ps_o.tile([128, TC], f32, tag="o", name="o_ps")
                    for hk in range(HPC):
                        nc.tensor.matmul(
                            ps, yT[hk][:, msl], wp_t[hk][:, n * TC:(n + 1) * TC],
                            start=(hk == 0), stop=(hk == HPC - 1),
                        )
                    ot = opool.tile([128, TC], f32, tag="ot", name="ot")
                    nc.scalar.copy(out=ot, in_=ps)
                    nc.sync.dma_start(out=out[msl, n * TC:(n + 1) * TC], in_=ot)

        for p in (opool, sm_pool, pp_pool, wppool, y_pool, v_pool,
                  qk_pool, mpool, ps_o, ps_d, ps_y, ps_s):
            p.release()
    nc.compile()
    return nc


def _host_prep(x, w_qkv, w_proj, freqs_cis):
    """Build per-core input maps (slicing + layout prep only)."""
    x = np.asarray(x, dtype=np.float32)
    w_qkv = np.asarray(w_qkv, dtype=np.float32)
    w_proj = np.asarray(w_proj, dtype=np.float32)
    fc = np.asarray(freqs_cis, dtype=np.float32)

    xTb = [np.ascontiguousarray(x[b].T) for b in range(B)]

    cos = fc[:, :, 0].T  # [64, T]
    sin = fc[:, :, 1].T
    cosP = np.repeat(cos, 2, axis=0).astype(np.float32)  # [128, T]
    sinP = np.repeat(sin, 2, axis=0).astype(np.float32)

    rt = np.zeros((HD, HD), dtype=np.float32)
    for d in range(HD // 2):
        rt[2 * d, 2 * d + 1] = 1.0
        rt[2 * d + 1, 2 * d] = -1.0

    masks = np.zeros((4, 128, TC), dtype=np.float32)
    ii = np.arange(TC)[None, :]
    jj = np.arange(128)[:, None]
    for d in range(4):
        masks[d] = (ii >= jj + 128 * d).astype(np.float32)

    ones = np.ones((128, 1), dtype=np.float32)

    in_maps = []
    for core in range(8):
        b = core // 4
        g = core % 4
        qc = np.ascontiguousarray(w_qkv[:, 512 * g: 512 * (g + 1)])
        kc = np.ascontiguousarray(w_qkv[:, 2048 + 512 * g: 2048 + 512 * (g + 1)])
        vc = np.ascontiguousarray(w_qkv[:, 4096 + 512 * g: 4096 + 512 * (g + 1)])
        wqk_c = np.concatenate([qc, kc], axis=1)
        wp_c = np.ascontiguousarray(w_proj[512 * g: 512 * (g + 1), :])
        in_maps.append({
            "xT": xTb[b],
            "wqk": wqk_c,
            "wv": vc,
            "wp": wp_c,
            "cosP": cosP,
            "sinP": sinP,
            "rt": rt,
            "masks": masks,
            "ones": ones,
        })
    return in_maps


def _get_nc():
    if "nc" not in _CACHE:
        _CACHE["nc"] = _build_nc()
    return _CACHE["nc"]


def kernel(x, w_qkv, w_proj, freqs_cis, attn_mask, _trace=False):
    from concourse.bass_utils import run_bass_kernel_spmd

    in_maps = _host_prep(x, w_qkv, w_proj, freqs_cis)
    nc = _get_nc()
    res = run_bass_kernel_spmd(
        nc, in_maps, core_ids=list(range(8)), trace=_trace,
    )
    outs = [r["out"].astype(np.float64) for r in res.results]
    full = np.stack([
        outs[0] + outs[1] + outs[2] + outs[3],
        outs[4] + outs[5] + outs[6] + outs[7],
    ]).astype(np.float32)
    if _trace:
        kernel._last_results = res
    return full



# revision 10
# speedup vs baseline: 1.2661x; 1.2661x over previous
"""Trainium2 Bass kernel for prefix-LM CausalSelfAttention (v2).

Problem: B=2, T=2048, C=2048, H=16 heads (hd=128), prefix-LM mask
(bidirectional over first half, causal after), RoPE on q/k.

Sharding over 8 cores: data-parallel on batch (2) x tensor-parallel on
heads (4 heads per core). Each core computes a partial output projection
(its heads' contribution); partials are summed on host.

v2 design (vs v1): bf16 data everywhere (validated 6.6e-3 rel err), x
resident in SBUF once (no second DMA pass), RoPE fused per-(m,chunk)
into stage A so DVE work hides under the QKV matmuls, attention exp
batched 2 key-tiles wide on ACT, softmax denominator via DVE-accumulated
pp sum + 4 tiny transposed matmuls + [128,4] reciprocal (replaces the
per-tile ones-matmuls and the 3.3us single-lane [1,512] reciprocal),
and the output projection interleaved into attention as PE filler.

Per-core dataflow:
  A. qkT[m] = W_{q,k}^T @ x^T per 512-chunk; RoPE combine per tile:
     rope = qkT*cos + (R @ qkT)*sin  (R = pair swap w/ sign)
  C. v[t-tile] = x @ Wv (natural layout)
  D. per (I, h): S'[j,i] tiles via k^T-tile x q-chunk, exp on ACT
     (2 tiles per ACTIVATE), pp accumulated on DVE for the denominator,
     PV accumulation into y^T psum; dT = pp_acc^T @ ones via 4 M=1
     matmuls, reciprocal, PE-transpose, gpsimd row broadcast, normalize.
  E. out[mt, n] = sum_hk yT[hk]^T @ Wp[hk], emitted as PE filler between
     attention batches; evacuation alternates ACT/DVE.
"""
import math

import numpy as np

N_HEAD = 16
B = 2
T = 2048
C = 2048
HD = 128
HPC = 4          # heads per core
CL = HPC * HD    # local C = 512
TC = 512         # chunk width (matmul moving free dim / psum bank)
NT = T // TC     # 4 chunks
KT = C // 128    # 16 contraction tiles over C
TT = T // 128    # 16 T tiles
SCALE = 1.0 / math.sqrt(HD)

# Per query-chunk I: batches of two 128-key tiles (j0, j0+1); mp indexes
# the two 1024-wide diagonal mask pairs, None for fully-allowed batches.
_BATCHES = {
    0: [(0, None), (2, None), (4, None), (6, None)],
    1: [(0, None), (2, None), (4, None), (6, None)],
    2: [(0, None), (2, None), (4, None), (6, None), (8, 0), (10, 1)],
    3: [(0, None), (2, None), (4, None), (6, None), (8, None), (10, None),
        (12, 0), (14, 1)],
}

_CACHE = {}


def _build_nc():
    from collections import deque

    import concourse.tile as tile
    import concourse.mybir as mybir
    from concourse import bacc

    f32 = mybir.dt.float32
    f32r = mybir.dt.float32r
    bf = mybir.dt.bfloat16
    Exp = mybir.ActivationFunctionType.Exp

    nc = bacc.Bacc(None, target_bir_lowering=False)

    xT = nc.dram_tensor("xT", [C, T], bf, kind="ExternalInput")
    wqk = nc.dram_tensor("wqk", [C, 2 * CL], bf, kind="ExternalInput")
    wv = nc.dram_tensor("wv", [C, CL], bf, kind="ExternalInput")
    wp = nc.dram_tensor("wp", [CL, C], bf, kind="ExternalInput")
    cosP = nc.dram_tensor("cosP", [HD, T], bf, kind="ExternalInput")
    sinP = nc.dram_tensor("sinP", [HD, T], bf, kind="ExternalInput")
    rt = nc.dram_tensor("rt", [HD, HD], bf, kind="ExternalInput")
    masks = nc.dram_tensor("masks", [2, 128, 2 * TC], bf, kind="ExternalInput")
    ones = nc.dram_tensor("ones", [128, 1], bf, kind="ExternalInput")
    ident = nc.dram_tensor("ident", [128, 128], f32, kind="ExternalInput")
    out = nc.dram_tensor("out", [T, C], f32, kind="ExternalOutput")

    xT3 = xT.rearrange("(kt p) t -> p kt t", p=128)
    wqk3 = wqk.rearrange("(kt p) m -> p kt m", p=128)
    wv3 = wv.rearrange("(kt p) m -> p kt m", p=128)
    wp3 = wp.rearrange("(hk p) m -> p hk m", p=128)
    masks3 = masks.rearrange("g p u -> p g u")

    with tile.TileContext(nc) as tc:
        # Left stack: mpool/rope (long-lived), then x (..stage C), then the
        # per-phase pools on top in LIFO order. Right stack: v/wp/yT which
        # outlive x. PSUM pools form their own stack.
        mpool = tc.alloc_tile_pool(name="misc", bufs=1)
        rope_pool = tc.alloc_tile_pool(name="rope", bufs=1)   # ..attention
        xpool = tc.alloc_tile_pool(name="x_sb", bufs=1)       # ..stage C

        rt_sb = mpool.tile([HD, HD], bf)
        ones_sb = mpool.tile([128, 1], bf)
        ident_sb = mpool.tile([128, 128], f32)
        mask_sb = mpool.tile([128, 2, 2 * TC], bf)
        warm_sb = mpool.tile([128, TC], bf)
        dume_sb = mpool.tile([128, 2], bf)

        # HAM warmup: PE matmuls on memset data while input DMAs stream,
        # so stage A starts at K=8/8. Also pre-trigger the exp table load.
        nc.vector.memset(warm_sb, 0.0)
        nc.scalar.activation(out=dume_sb, in_=warm_sb[:, 0:2], func=Exp)
        ps_w = tc.alloc_tile_pool(name="ps_warm", bufs=1, space="PSUM")
        for _ in range(26):
            pw = ps_w.tile([128, TC], f32, tag="pw", name="pw")
            nc.tensor.matmul(pw, warm_sb[:, 0:128], warm_sb, start=True,
                             stop=True)
        ps_w.release()

        # ---- input DMAs (sync-queue order = arrival order) ----
        wpool = tc.alloc_tile_pool(name="wqk_sb", bufs=1)     # ..stage A
        tpool = tc.alloc_tile_pool(name="trig", bufs=1)       # ..stage A
        qk_pool = tc.alloc_tile_pool(name="qk", bufs=1)       # ..stage A

        w_t = []
        x_t = {}
        for k in range(KT):
            wt = wpool.tile([128, 2 * CL], bf, tag=f"w{k}", name=f"w{k}")
            nc.sync.dma_start(out=wt, in_=wqk3[:, k])
            w_t.append(wt)
            xt = xpool.tile([128, TC], bf, tag=f"x{k}_0", name=f"x{k}_0")
            nc.sync.dma_start(out=xt, in_=xT3[:, k, 0:TC])
            x_t[(k, 0)] = xt
        cos_sb = tpool.tile([HD, T], bf)
        sin_sb = tpool.tile([HD, T], bf)
        nc.sync.dma_start(out=rt_sb, in_=rt[:, :])
        nc.sync.dma_start(out=ident_sb, in_=ident[:, :])
        nc.sync.dma_start(out=ones_sb, in_=ones[:, :])
        nc.sync.dma_start(out=cos_sb, in_=cosP[:, :])
        nc.sync.dma_start(out=sin_sb, in_=sinP[:, :])
        for n in range(1, NT):
            for k in range(KT):
                xt = xpool.tile([128, TC], bf, tag=f"x{k}_{n}",
                                name=f"x{k}_{n}")
                nc.sync.dma_start(out=xt, in_=xT3[:, k, n * TC:(n + 1) * TC])
                x_t[(k, n)] = xt

        # ---- stage A: qkT + fused RoPE ----
        ps_a = tc.alloc_tile_pool(name="ps_a", bufs=4, space="PSUM")
        ps_r = tc.alloc_tile_pool(name="ps_rot", bufs=2, space="PSUM")
        rtmp = tc.alloc_tile_pool(name="rope_tmp", bufs=1)

        qk_sb = [qk_pool.tile([128, T], bf, tag=f"qk{m}", name=f"qk{m}")
                 for m in range(8)]
        rope_sb = [rope_pool.tile([128, T], bf, tag=f"ro{m}", name=f"ro{m}")
                   for m in range(8)]

        def emit_rope(m, nsl):
            # R @ qk on PE (pair swap w/ sign), combine on DVE in bf16.
            psr = ps_r.tile([128, TC], f32, tag="ps_r", name="ps_r")
            nc.tensor.matmul(psr, rt_sb, qk_sb[m][:, nsl], start=True,
                             stop=True)
            t1 = rtmp.tile([128, TC], bf, tag="t1", name="t1", bufs=2)
            nc.vector.tensor_mul(t1, psr, sin_sb[:, nsl])
            t2 = rtmp.tile([128, TC], bf, tag="t2", name="t2", bufs=2)
            nc.vector.tensor_mul(t2, qk_sb[m][:, nsl], cos_sb[:, nsl])
            nc.vector.tensor_add(rope_sb[m][:, nsl], t1, t2)

        pending_rope = None
        for n in range(NT):
            nsl = slice(n * TC, (n + 1) * TC)
            for m in range(8):
                ps = ps_a.tile([128, TC], f32, tag="ps_a", name="ps_a")
                for k in range(KT):
                    nc.tensor.matmul(ps, w_t[k][:, m * 128:(m + 1) * 128],
                                     x_t[(k, n)],
                                     start=(k == 0), stop=(k == KT - 1))
                nc.scalar.copy(out=qk_sb[m][:, nsl], in_=ps)
                # rope of the PREVIOUS tile: its ACT copy finished during
                # this group's 16 matmuls, so the R-matmul never stalls PE.
                if pending_rope is not None:
                    emit_rope(*pending_rope)
                pending_rope = (m, nsl)
        emit_rope(*pending_rope)

        rtmp.release()
        ps_r.release()
        ps_a.release()
        qk_pool.release()
        tpool.release()
        wpool.release()

        # ---- stage C: v = x @ Wv; wp/masks DMAs land during this phase ----
        wvpool = tc.alloc_tile_pool(name="wv_sb", bufs=1)     # ..stage C
        v_pool = tc.alloc_tile_pool(name="v_sb", bufs=1, side="right")
        wppool = tc.alloc_tile_pool(name="wp_sb", bufs=1, side="right")

        wv_t = []
        for k in range(KT):
            wt = wvpool.tile([128, CL], bf, tag=f"wv{k}", name=f"wv{k}")
            nc.sync.dma_start(out=wt, in_=wv3[:, k])
            wv_t.append(wt)
        wp_t = []
        for hk in range(HPC):
            wt = wppool.tile([128, C], bf, tag=f"wp{hk}", name=f"wp{hk}")
            nc.sync.dma_start(out=wt, in_=wp3[:, hk])
            wp_t.append(wt)
        nc.sync.dma_start(out=mask_sb, in_=masks3)

        v_t = [v_pool.tile([128, CL], bf, tag=f"v{mt}", name=f"v{mt}")
               for mt in range(TT)]
        ps_c = tc.alloc_tile_pool(name="ps_c", bufs=4, space="PSUM")
        for mt in range(TT):
            ps = ps_c.tile([128, CL], f32, tag="ps_c", name="ps_c")
            n, off = mt // 4, (mt % 4) * 128
            for k in range(KT):
                nc.tensor.matmul(ps, x_t[(k, n)][:, off:off + 128], wv_t[k],
                                 start=(k == 0), stop=(k == KT - 1))
            nc.scalar.copy(out=v_t[mt], in_=ps)
        ps_c.release()
        wvpool.release()
        xpool.release()

        # ---- stage D attention + stage E (proj) as PE filler ----
        y_pool = tc.alloc_tile_pool(name="yT_sb", bufs=1, side="right")
        yT = [y_pool.tile([128, T], bf, tag=f"yT{h}", name=f"yT{h}")
              for h in range(HPC)]

        pp_pool = tc.alloc_tile_pool(name="pp", bufs=1)
        acc_pool = tc.alloc_tile_pool(name="accp", bufs=1)
        sm_pool = tc.alloc_tile_pool(name="small", bufs=1)
        o_pool = tc.alloc_tile_pool(name="ostage", bufs=1)
        ps_s = tc.alloc_tile_pool(name="ps_s", bufs=2, space="PSUM")
        ps_y = tc.alloc_tile_pool(name="ps_y", bufs=1, space="PSUM")
        ps_d = tc.alloc_tile_pool(name="ps_d", bufs=1, space="PSUM")
        ps_o = tc.alloc_tile_pool(name="ps_o", bufs=1, space="PSUM")

        e_jobs = deque()
        e_count = [0]

        def emit_e_group():
            if not e_jobs:
                return
            mt, n2 = e_jobs.popleft()
            msl = slice(mt * 128, (mt + 1) * 128)
            nsl = slice(n2 * TC, (n2 + 1) * TC)
            pso = ps_o.tile([128, TC], f32, tag="o", name="o_ps")
            for hk in range(HPC):
                nc.tensor.matmul(pso, yT[hk][:, msl], wp_t[hk][:, nsl],
                                 start=(hk == 0), stop=(hk == HPC - 1))
            ot = o_pool.tile([128, TC], f32, tag="ot", name="ot", bufs=3)
            # alternate evacuation engine to balance ACT vs DVE load
            if e_count[0] % 2 == 0:
                nc.scalar.copy(out=ot, in_=pso)
            else:
                nc.vector.tensor_copy(out=ot, in_=pso)
            e_count[0] += 1
            nc.sync.dma_start(out=out[msl, nsl], in_=ot)

        for I in range(NT):
            isl = slice(I * TC, (I + 1) * TC)
            for h in range(HPC):
                q_h = rope_sb[h]
                k_h = rope_sb[4 + h]
                bt = _BATCHES[I]
                nb = len(bt)
                acc = acc_pool.tile([128, TC], bf, tag="acc", name="acc",
                                    bufs=2)
                y_ps = ps_y.tile([128, TC], f32, tag="y", name="y_ps")
                for bi, (j0, mp) in enumerate(bt):
                    s_ps = ps_s.tile([128, 2 * TC], f32, tag="s", name="s_ps")
                    for half in range(2):
                        J = j0 + half
                        nc.tensor.matmul(
                            s_ps[:, half * TC:(half + 1) * TC],
                            k_h[:, J * 128:(J + 1) * 128], q_h[:, isl],
                            start=True, stop=True, skip_group_check=True,
                        )
                    pp = pp_pool.tile([128, 2 * TC], bf, tag="pp", name="pp",
                                      bufs=2)
                    nc.scalar.activation(out=pp, in_=s_ps, func=Exp,
                                         scale=SCALE)
                    src = pp
                    if mp is not None:
                        ppm = pp_pool.tile([128, 2 * TC], bf, tag="ppm",
                                           name="ppm", bufs=2)
                        nc.vector.tensor_mul(ppm, pp, mask_sb[:, mp])
                        src = ppm
                    if bi == 0:
                        nc.vector.tensor_copy(out=acc, in_=src[:, 0:TC])
                    else:
                        nc.vector.tensor_add(acc, acc, src[:, 0:TC])
                    nc.vector.tensor_add(acc, acc, src[:, TC:2 * TC])
                    for half in range(2):
                        J = j0 + half
                        nc.tensor.matmul(
                            y_ps, v_t[J][:, h * 128:(h + 1) * 128],
                            src[:, half * TC:(half + 1) * TC],
                            start=(bi == 0 and half == 0),
                            stop=(bi == nb - 1 and half == 1),
                        )
                    emit_e_group()
                # denominator (transposed layout) + normalize
                d_ps = ps_d.tile([128, 4], f32, tag="d", name="d_ps")
                for qq in range(4):
                    nc.tensor.matmul(d_ps[:, qq:qq + 1],
                                     acc[:, qq * 128:(qq + 1) * 128],
                                     ones_sb, start=True, stop=True,
                                     skip_group_check=True)
                recip = sm_pool.tile([128, 4], f32, tag="recip",
                                     name="recip", bufs=2)
                nc.vector.reciprocal(out=recip, in_=d_ps)
                # 4 column transposes into one [1, 512] psum row so the
                # gpsimd broadcast reads from partition 0 in one shot
                tT_ps = ps_d.tile([1, TC], f32, tag="tT", name="tT_ps")
                for qq in range(4):
                    nc.tensor.transpose(tT_ps[:, qq * 128:(qq + 1) * 128],
                                        recip[:, qq:qq + 1], ident_sb)
                recipT = sm_pool.tile([1, TC], f32, tag="recipT",
                                      name="recipT", bufs=2)
                nc.vector.tensor_copy(out=recipT, in_=tT_ps)
                recipB = sm_pool.tile([128, TC], f32, tag="recipB",
                                      name="recipB", bufs=2)
                nc.gpsimd.partition_broadcast(recipB, recipT)
                nc.vector.tensor_mul(yT[h][:, isl], y_ps, recipB)
            for ml in range(4):
                for n2 in range(NT):
                    e_jobs.append((4 * I + ml, n2))
        while e_jobs:
            emit_e_group()

        for p in (o_pool, sm_pool, acc_pool, pp_pool, rope_pool, mpool,
                  y_pool, wppool, v_pool, ps_o, ps_d, ps_y, ps_s):
            p.release()
    nc.compile()
    return nc


def _host_prep(x, w_qkv, w_proj, freqs_cis):
    """Build per-core input maps (slicing + layout + dtype prep only)."""
    import ml_dtypes
    BF = ml_dtypes.bfloat16

    x = np.asarray(x, dtype=np.float32)
    w_qkv = np.asarray(w_qkv, dtype=np.float32)
    w_proj = np.asarray(w_proj, dtype=np.float32)
    fc = np.asarray(freqs_cis, dtype=np.float32)

    xTb = [np.ascontiguousarray(x[b].T).astype(BF) for b in range(B)]

    cos = fc[:, :, 0].T  # [64, T]
    sin = fc[:, :, 1].T
    cosP = np.repeat(cos, 2, axis=0).astype(BF)  # [128, T]
    sinP = np.repeat(sin, 2, axis=0).astype(BF)

    rt = np.zeros((HD, HD), dtype=np.float32)
    for d in range(HD // 2):
        rt[2 * d, 2 * d + 1] = 1.0
        rt[2 * d + 1, 2 * d] = -1.0
    rt = rt.astype(BF)

    # masks[p][jj, u]: pair p covers diagonal tiles d = 2p + u//TC
    masks = np.zeros((2, 128, 2 * TC), dtype=np.float32)
    jj = np.arange(128)[:, None]
    for p in range(2):
        for tp in range(2):
            d = 2 * p + tp
            ii = np.arange(TC)[None, :]
            masks[p][:, tp * TC:(tp + 1) * TC] = (ii >= jj + 128 * d)
    masks = masks.astype(BF)

    ones = np.ones((128, 1), dtype=np.float32).astype(BF)
    ident = np.eye(128, dtype=np.float32)

    in_maps = []
    for core in range(8):
        b = core // 4
        g = core % 4
        qc = w_qkv[:, 512 * g: 512 * (g + 1)]
        kc = w_qkv[:, 2048 + 512 * g: 2048 + 512 * (g + 1)]
        vc = w_qkv[:, 4096 + 512 * g: 4096 + 512 * (g + 1)]
        wqk_c = np.concatenate([qc, kc], axis=1).astype(BF)
        wv_c = np.ascontiguousarray(vc).astype(BF)
        wp_c = np.ascontiguousarray(
            w_proj[512 * g: 512 * (g + 1), :]).astype(BF)
        in_maps.append({
            "xT": xTb[b],
            "wqk": wqk_c,
            "wv": wv_c,
            "wp": wp_c,
            "cosP": cosP,
            "sinP": sinP,
            "rt": rt,
            "masks": masks,
            "ones": ones,
            "ident": ident,
        })
    return in_maps


def _get_nc():
    if "nc" not in _CACHE:
        _CACHE["nc"] = _build_nc()
    return _CACHE["nc"]


def kernel(x, w_qkv, w_proj, freqs_cis, attn_mask, _trace=False):
    from concourse.bass_utils import run_bass_kernel_spmd

    in_maps = _host_prep(x, w_qkv, w_proj, freqs_cis)
    nc = _get_nc()
    res = run_bass_kernel_spmd(
        nc, in_maps, core_ids=list(range(8)), trace=_trace,
    )
    outs = [r["out"].astype(np.float64) for r in res.results]
    full = np.stack([
        outs[0] + outs[1] + outs[2] + outs[3],
        outs[4] + outs[5] + outs[6] + outs[7],
    ]).astype(np.float32)
    if _trace:
        kernel._last_results = res
    return full
